# revision 1
# baseline (speedup 1.0000x reference)
"""Trainium2 Bass kernel for a 2-layer GAT-style reduction network.

Reference math (per head h, per group of 16 nodes):
    wx   = x @ W[h]                                  # [*, n, d]
    z    = gelu(wx @ A_top[h] + wx[root] @ A_bot[h]) # root = node 0 of group
    att  = softmax(gelu(z), over n)
    out_h[g] = gelu(sum_n att[n] * wx[n])
    layer out = mean_h out_h
Applied twice: layer0 groups = N1(16) within (b, n2); layer1 groups = N2(16)
within b.

Key algebraic restructure used here:
  - sum_n att[n]*(x[n] @ W) == (sum_n att[n]*x[n]) @ W, so the big matmul
    runs per-group (1024 rows/core) instead of per-node (16384 rows/core).
  - wx @ A_top == x @ (W @ A_top) := x @ a_eff, so attention scores come from
    a thin [F, 8] matmul (a_eff/b_eff for 4 heads), not from wx.
  - head-mean of layer0 is absorbed into layer1 weights (0.25 * W1/aeff1);
    final 0.25 applied explicitly.

Distribution: pure data-parallel over 8 NeuronCores, sharding the batch
(512 -> 64 per core). Weights replicated. No collectives; host concatenates
the 8 output shards. x is cast to bf16 on the host (halves the HBM load).

Dataflow per core (layer 0), software-pipelined 4 deep — emission order
staggers B(c), C(c-1), D(c-2), E(c-3) so every in-order engine queue holds
independent next-chunk work ahead of each chunk's dependent tail:
  B: DMA x blocks (1024 tok, bf16) HBM->SBUF on alternating queues;
     PE-transpose x tiles (4 per PSUM bank per accumulation group) ->
     xtb [f, t] bf16 via one DVE copy per bank; thin zall matmul (a_eff)
     -> z scores [8, 1024] PSUM per block; one ACT drain; SWDGE-pack into
     zbig [16, 2048] (row 4*bi+h, a|b on column halves).
  C: z epilogue on [16, 1024]: z = za + zb_root (broadcast AP), gelu, gelu
     (Cg), then exp, segmented sum (16), reciprocal, att = e * recip (Cx).
  D: att rows DMA-repacked to base-0, PE-transposed to [t, h]; S_att
     [t, (h,g)] = att * group-mask (one DVE op per block); stage-1 matmul
     with x tile stationary -> y^T accumulates in PSUM, drained to ybuf.
  E: stage-2 matmul W^T @ y^T (N=512) -> gelu -> head-sum -> x1^T; layer-1
     prep hoisted per chunk (PE re-transpose of x1 cols, a/b-split zall1
     into partition-aligned zL1[h, a|b, t]).
Layer 1: epilogue straight off zL1 (no packing), att lands directly in the
stage-1 layout; stage-1/stage-2 as in layer 0 on 1024 tokens.

Avoid: SBUF->SBUF `transpose=True` DMAs (~300us each on HW — element
descriptors) and partition-crossing multi-dim rearranges in DMA APs
(silently scramble data).
"""

import sys

sys.path.insert(0, "/opt/trn_rl_repo")

import numpy as np
from contextlib import ExitStack

import concourse.bass as bass
import concourse.tile as tile
from concourse import bacc, mybir
from concourse.bass_utils import run_bass_kernel_spmd

dt = mybir.dt
AF = mybir.ActivationFunctionType

NCORES = 8
B, N2, N1, F, D, H = 512, 16, 16, 256, 256, 4
BS = B // NCORES  # 64 samples per core
T0 = BS * N2 * N1  # 16384 tokens, layer 0
NB0 = T0 // 1024  # 16 blocks of 1024 tokens
NCH = 4  # super-chunks (4 blocks each)
CB = NB0 // NCH  # 8 blocks per chunk
T1 = BS * N2  # 1024 tokens, layer 1
G0 = T0 // 16  # 1024 groups layer 0
G1 = T1 // 16  # 64 groups layer 1

import os as _os

REPS = int(_os.environ.get("KREPS", "1"))
TRACE = False  # set True (e.g. from test.py) to capture an NTFF profile
DEBUG = False  # add intermediate dram outputs for stage-wise HW debugging
_CACHE = {}


def f32(ap):
    return ap.bitcast(dt.float32)


def build_program():
    nc = bacc.Bacc("TRN2", target_bir_lowering=False, debug=False)

    x_d = nc.dram_tensor("x", [T0, F], dt.bfloat16, kind="ExternalInput").ap()
    w0_d = nc.dram_tensor("w0", [H, F, D], dt.float32r, kind="ExternalInput").ap()
    w1_d = nc.dram_tensor("w1", [H, D, D], dt.float32r, kind="ExternalInput").ap()
    aeb_d = nc.dram_tensor("aeb", [2, 128, 8], dt.bfloat16, kind="ExternalInput").ap()
    ae1_d = nc.dram_tensor("ae1", [2, 128, 8], dt.float32r, kind="ExternalInput").ap()
    mm_d = nc.dram_tensor("mmask", [128, 32], dt.float32, kind="ExternalInput").ap()
    id4_d = nc.dram_tensor("id4", [4, 4], dt.float32r, kind="ExternalInput").ap()
    id128_d = nc.dram_tensor("id128", [128, 128], dt.float32r, kind="ExternalInput").ap()
    id128b_d = nc.dram_tensor("id128b", [128, 128], dt.bfloat16, kind="ExternalInput").ap()
    out_d = nc.dram_tensor("out", [BS, D], dt.float32, kind="ExternalOutput").ap()
    if DEBUG:
        dbg_zbig = nc.dram_tensor("dbg_zbig", [64, 1024], dt.float32, kind="ExternalOutput").ap()
        dbg_att = nc.dram_tensor("dbg_att", [64, 512], dt.float32, kind="ExternalOutput").ap()
        dbg_ybuf = nc.dram_tensor("dbg_ybuf", [128, 2, 2048], dt.float32, kind="ExternalOutput").ap()
        dbg_x1t = nc.dram_tensor("dbg_x1t", [128, 2, 1024], dt.float32, kind="ExternalOutput").ap()
        dbg_xtb = nc.dram_tensor("dbg_xtb", [128, 2, 512], dt.bfloat16, kind="ExternalOutput").ap()
        dbg_zt0 = nc.dram_tensor("dbg_zt0", [8, 512], dt.float32, kind="ExternalOutput").ap()

    with tile.TileContext(nc) as tc, ExitStack() as ctx:
        cpool = ctx.enter_context(tc.tile_pool(name="consts", bufs=1))
        ztpool = ctx.enter_context(tc.tile_pool(name="zt", bufs=3))
        xbpool = ctx.enter_context(tc.tile_pool(name="xb", bufs=12))
        xtpool = ctx.enter_context(tc.tile_pool(name="xt", bufs=3))
        zbpool = ctx.enter_context(tc.tile_pool(name="zbig", bufs=2))
        epool = ctx.enter_context(tc.tile_pool(name="eps", bufs=2))
        atpool = ctx.enter_context(tc.tile_pool(name="att", bufs=2))
        abpool = ctx.enter_context(tc.tile_pool(name="attb", bufs=2))
        sapool = ctx.enter_context(tc.tile_pool(name="sab", bufs=3))
        ybpool = ctx.enter_context(tc.tile_pool(name="ybuf", bufs=2))
        ghpool = ctx.enter_context(tc.tile_pool(name="gh", bufs=4))
        adpool = ctx.enter_context(tc.tile_pool(name="ad", bufs=2))
        x1pool = ctx.enter_context(tc.tile_pool(name="x1", bufs=1))
        mpool = ctx.enter_context(tc.tile_pool(name="misc", bufs=1))

        ps_z = ctx.enter_context(tc.tile_pool(name="ps_z", bufs=1, space="PSUM"))
        ps_at = ctx.enter_context(tc.tile_pool(name="ps_at", bufs=2, space="PSUM"))
        ps_s1 = ctx.enter_context(tc.tile_pool(name="ps_s1", bufs=2, space="PSUM"))
        ps_s2 = ctx.enter_context(tc.tile_pool(name="ps_s2", bufs=2, space="PSUM"))

        # ---- constants ----
        # w0/w1 (2 MB, first needed by phase E ~40us in) go on the otherwise
        # SWDGE (gpsimd) queue so chunk 0's x loads start immediately on sync.
        w0_t = cpool.tile([128, H, 2, D], dt.float32r, tag="w0")
        w1_t = cpool.tile([128, H, 2, D], dt.float32r, tag="w1")
        aeb_t = cpool.tile([128, 2, 8], dt.bfloat16, tag="aeb")
        nc.sync.dma_start(out=aeb_t[:], in_=aeb_d.rearrange("s p j -> p s j"))
        ae1_t = cpool.tile([128, 2, 8], dt.float32r, tag="ae1")
        nc.sync.dma_start(out=ae1_t[:], in_=ae1_d.rearrange("s p j -> p s j"))
        mm_t = cpool.tile([128, 32], dt.float32, tag="mm")
        nc.sync.dma_start(out=mm_t[:], in_=mm_d)
        id4_t = cpool.tile([4, 4], dt.float32r, tag="id4")
        nc.sync.dma_start(out=id4_t[:], in_=id4_d)
        id128_t = cpool.tile([128, 128], dt.float32r, tag="id128")
        nc.sync.dma_start(out=id128_t[:], in_=id128_d)
        id128b_t = cpool.tile([128, 128], dt.bfloat16, tag="id128b")
        nc.sync.dma_start(out=id128b_t[:], in_=id128b_d)


        for rep in range(REPS):
            # ================= LAYER 0 =================
            # Per-chunk pipeline. DMA queue assignment (queues are FIFO per
            # engine): sync = x loads + transposed reads + output; scalar =
            # bounce writes; gpsimd = small packing DMAs.
            x1T1 = x1pool.tile([128, 2, 1024], dt.float32r, tag="x1T", name="x1T")
            x1n_t = x1pool.tile([128, 8, 256], dt.float32r, tag="x1n", name="x1n")
            zL1 = x1pool.tile([4, 2, 1024], dt.float32, tag="zL1", name="zL1")
            attb1 = abpool.tile([4, 1024], dt.float32r, tag="attb1", bufs=1)
            y1b = [
                mpool.tile([128, 256], dt.float32r, tag=f"y1b{ds}", name=f"y1b{ds}") for ds in range(2)
            ]
            def emit_B(c):
                # ---- phase A+B: load, transpose, zall, pack (from PSUM) ----
                zbig = zbpool.tile([16, 2048], dt.float32, tag="zbig", name="zbig")
                xbs = {}
                for bi in range(CB):
                    b = c * CB + bi
                    xb = xbpool.tile([128, 8, F], dt.bfloat16, tag="xb", name="xb")
                    qeng = nc.sync if b % 2 == 0 else nc.scalar
                    qeng.dma_start(
                        out=xb[:],
                        in_=x_d[1024 * b : 1024 * (b + 1), :].rearrange(
                            "(n p) f -> p n f", p=128
                        ),
                    )
                    xbs[b] = xb
                    xtb = xtpool.tile([128, 2, 1024], dt.bfloat16, tag="xt", name="xtb")
                    # PE-transpose x tiles (4 per PSUM bank as one accumulation
                    # group), then one DVE copy per bank -> xtb bf16.
                    for fs in range(2):
                        for half in range(2):
                            tp = ps_at.tile([128, 512], dt.bfloat16, tag="atp", name="tp")
                            for kk in range(4):
                                k = 4 * half + kk
                                nc.tensor.matmul(
                                    tp[:, 128 * kk : 128 * (kk + 1)],
                                    xb[:, k, 128 * fs : 128 * (fs + 1)],
                                    id128b_t[:],
                                    is_transpose=True,
                                    start=(kk == 0),
                                    stop=(kk == 3),
                                )
                            nc.vector.tensor_copy(
                                xtb[:, fs, 512 * half : 512 * (half + 1)], tp[:]
                            )
                    # z scores: block pairs share one [40, 1024] PSUM tile
                    # (rows 0:8 and 32:40 — base-32 is a legal matmul output
                    # offset); one drain + two pack DMAs per pair.
                    if bi % 2 == 0:
                        zpair = ps_z.tile([40, 1024], dt.float32, tag="zps", name="zps")
                    zoff = 32 * (bi % 2)
                    for half in range(2):
                        for fs in range(2):
                            nc.tensor.matmul(
                                zpair[zoff : zoff + 8, 512 * half : 512 * (half + 1)],
                                aeb_t[:, fs, :],
                                xtb[:, fs, 512 * half : 512 * (half + 1)],
                                start=(fs == 0),
                                stop=(fs == 1),
                            )
                    if bi % 2 == 1:
                        zt = ztpool.tile([40, 1024], dt.float32, tag="zt", name="zt")
                        nc.scalar.copy(zt[:], zpair[:])
                        for q in range(2):
                            bb = bi - 1 + q
                            nc.gpsimd.dma_start(
                                out=zbig[4 * bb : 4 * bb + 4, 0:1024],
                                in_=zt[32 * q : 32 * q + 4, :],
                            )
                            nc.gpsimd.dma_start(
                                out=zbig[4 * bb : 4 * bb + 4, 1024:2048],
                                in_=zt[32 * q + 4 : 32 * q + 8, :],
                            )
                    if DEBUG and b == 0:
                        nc.sync.dma_start(out=dbg_xtb, in_=xtb[:, :, 0:512])
                if DEBUG and c == 0:
                    nc.sync.dma_start(out=dbg_zbig, in_=zbig[:])
                return {"xbs": xbs, "zbig": zbig}

            def emit_Cg(c, st):
                zbig = st["zbig"]
                # ---- phase C (gelu part): z -> gelu(gelu(z)) ----
                zs = epool.tile([16, 1024], dt.float32, tag="epsA", name="zs")
                nc.vector.tensor_add(
                    zs[:].rearrange("p (g j) -> p g j", j=16),
                    zbig[:, 0:1024].rearrange("p (g j) -> p g j", j=16),
                    zbig[:, 1024:2048]
                    .rearrange("p (g j) -> p g j", j=16)[:, :, 0:1]
                    .broadcast_to([16, 64, 16]),
                )
                g1 = epool.tile([16, 1024], dt.float32, tag="epsB", name="g1")
                nc.scalar.activation(g1[:], zs[:], AF.Gelu)
                sv = epool.tile([16, 1024], dt.float32, tag="epsS", name="sv", bufs=1)
                nc.scalar.activation(sv[:], g1[:], AF.Gelu)
                st["sv"] = sv

            def emit_Cx(c, st):
                # ---- phase C (exp part): softmax over groups of 16 ----
                sv = st.pop("sv")
                e = epool.tile([16, 1024], dt.float32, tag="epsB", name="e")
                nc.scalar.activation(e[:], sv[:], AF.Exp)
                den = mpool.tile([16, 64], dt.float32, tag=f"den{c}", name="den")
                nc.vector.reduce_sum(
                    den[:].unsqueeze(2),
                    e[:].rearrange("p (g j) -> p g j", j=16),
                    axis=mybir.AxisListType.X,
                )
                rec = mpool.tile([16, 64], dt.float32, tag=f"rec{c}", name="rec")
                nc.vector.reciprocal(rec[:], den[:])
                att = atpool.tile([16, 1024], dt.float32r, tag="att", name="att")
                nc.vector.tensor_mul(
                    att[:].rearrange("p (g j) -> p g j", j=16),
                    e[:].rearrange("p (g j) -> p g j", j=16),
                    rec[:].unsqueeze(2).broadcast_to([16, 64, 16]),
                )
                if DEBUG and c == 0:
                    nc.sync.dma_start(out=dbg_att, in_=f32(att[:]))
                st["att"] = att

            def emit_D(c, st):
                xbs, att = st["xbs"], st["att"]
                # ---- phase D: att transpose + S_att + stage-1 (bf16) ----
                ybuf = ybpool.tile([128, 2, 1024], dt.float32r, tag="ybuf", name="ybuf")
                ybps = [None, None]
                for bp in range(CB):
                    b = c * CB + bp
                    attb = abpool.tile([4, 1024], dt.float32r, tag="attb", name="attb")
                    nc.sync.dma_start(out=attb[:], in_=att[4 * bp : 4 * bp + 4, :])
                    atp = ps_at.tile([128, 32], dt.float32r, tag="atp", name="atp")
                    for k in range(8):
                        nc.tensor.transpose(
                            atp[:, 4 * k : 4 * k + 4],
                            attb[0:4, 128 * k : 128 * (k + 1)],
                            id4_t[:],
                        )
                    sab = sapool.tile([128, 8, 32], dt.bfloat16, tag="sab", name="sab")
                    nc.vector.tensor_mul(
                        sab[:].rearrange("p k (h g) -> p k h g", g=8),
                        f32(atp[:])
                        .rearrange("p (k h) -> p k h", h=4)
                        .unsqueeze(3)
                        .broadcast_to([128, 8, 4, 8]),
                        mm_t[:]
                        .rearrange("p (h g) -> p h g", g=8)
                        .unsqueeze(1)
                        .broadcast_to([128, 8, 4, 8]),
                    )
                    for k in range(8):
                        K = b * 8 + k  # global x-tile index
                        kq = K % 16  # position within psum group
                        if kq == 0:
                            ybps = [
                                ps_s1.tile([128, 512], dt.float32, tag="ybps", name="ybps")
                                for _ in range(2)
                            ]
                        for fs in range(2):
                            nc.tensor.matmul(
                                ybps[fs][:, 32 * kq : 32 * kq + 32],
                                xbs[b][:, k, 128 * fs : 128 * (fs + 1)],
                                sab[:, k, :],
                                start=(kq == 0),
                                stop=(kq == 15),
                            )
                        if kq == 15:
                            q = (K % 32) // 16
                            nc.scalar.copy(ybuf[:, 0, 512 * q : 512 * (q + 1)], ybps[0][:])
                            nc.vector.tensor_copy(ybuf[:, 1, 512 * q : 512 * (q + 1)], ybps[1][:])
                if DEBUG and c == 0:
                    nc.sync.dma_start(out=dbg_ybuf, in_=f32(ybuf[:]))
                st["ybuf"] = ybuf

            def emit_E(c, st):
                ybuf = st["ybuf"]
                # ---- phase E: stage-2 + gelu head-sum ----
                # Both ds halves share one [128, 512] PSUM bank as a single
                # striped accumulation group -> one gelu per head.
                ghs = []
                for h in range(H):
                    o2 = ps_s2.tile([128, 512], dt.float32, tag="o2", name="o2")
                    for ds in range(2):
                        for fs in range(2):
                            nc.tensor.matmul(
                                o2[:, 256 * ds : 256 * (ds + 1)],
                                w0_t[:, h, fs, 128 * ds : 128 * (ds + 1)],
                                ybuf[:, fs, :].rearrange(
                                    "p (K hh g) -> p K hh g", hh=4, g=8
                                )[:, :, h, :],
                                start=(ds == 0 and fs == 0),
                                stop=(ds == 1 and fs == 1),
                            )
                    gh = ghpool.tile([128, 512], dt.float32, tag="gh", name="gh")
                    nc.scalar.activation(gh[:], o2[:], AF.Gelu)
                    ghs.append(gh)
                ad1 = adpool.tile([128, 512], dt.float32, tag="ad", name="ad1")
                nc.vector.tensor_add(ad1[:], ghs[0][:], ghs[1][:])
                ad2 = adpool.tile([128, 512], dt.float32, tag="ad", name="ad2")
                nc.vector.tensor_add(ad2[:], ghs[2][:], ghs[3][:])
                nc.vector.tensor_add(
                    x1T1[:, :, 256 * c : 256 * (c + 1)],
                    ad1[:].rearrange("p (ds i) -> p ds i", ds=2),
                    ad2[:].rearrange("p (ds i) -> p ds i", ds=2),
                )

                # hoisted layer-1 prep: transpose this chunk's x1T cols; zall1
                # once a 512-col half completes (split a/b so the epilogue is
                # partition-aligned: zL1[h, 0, t]=a-score, zL1[h, 1, t]=b-score)
                for j in (2 * c, 2 * c + 1):
                    for ds in range(2):
                        trp = ps_at.tile([128, 128], dt.float32r, tag="atp", name="trp")
                        nc.tensor.transpose(
                            trp[:], x1T1[:, ds, 128 * j : 128 * (j + 1)], id128_t[:]
                        )
                        nc.vector.tensor_copy(
                            x1n_t[:, j, 128 * ds : 128 * (ds + 1)], f32(trp[:])
                        )
                if c % 2 == 1:
                    lb = c // 2
                    sl = slice(512 * lb, 512 * (lb + 1))
                    for ab in range(2):
                        z1p = ps_z.tile([4, 512], dt.float32, tag="zps", name="z1p")
                        for ds in range(2):
                            nc.tensor.matmul(
                                z1p[:],
                                ae1_t[:, ds, 4 * ab : 4 * ab + 4],
                                x1T1[:, ds, sl],
                                start=(ds == 0),
                                stop=(ds == 1),
                            )
                        nc.scalar.copy(zL1[:, ab, sl], z1p[:])

            def emit_L1half(lb):
                sl = slice(512 * lb, 512 * (lb + 1))
                # L1 half-epilogue: groups of 16 are complete per half
                zs1h = epool.tile([4, 512], dt.float32, tag="epsA", name="zs1h")
                nc.vector.tensor_add(
                    zs1h[:].rearrange("p (g j) -> p g j", j=16),
                    zL1[:, 0, sl].rearrange("p (g j) -> p g j", j=16),
                    zL1[:, 1, sl]
                    .rearrange("p (g j) -> p g j", j=16)[:, :, 0:1]
                    .broadcast_to([4, 32, 16]),
                )
                g11h = epool.tile([4, 512], dt.float32, tag="epsB", name="g11h")
                nc.scalar.activation(g11h[:], zs1h[:], AF.Gelu)
                s1th = epool.tile([4, 512], dt.float32, tag="epsS", name="s1th", bufs=1)
                nc.scalar.activation(s1th[:], g11h[:], AF.Gelu)
                e1h = epool.tile([4, 512], dt.float32, tag="epsB", name="e1h")
                nc.scalar.activation(e1h[:], s1th[:], AF.Exp)
                den1h = mpool.tile([4, 32], dt.float32, tag=f"den1h{lb}")
                nc.vector.reduce_sum(
                    den1h[:].unsqueeze(2),
                    e1h[:].rearrange("p (g j) -> p g j", j=16),
                    axis=mybir.AxisListType.X,
                )
                rec1h = mpool.tile([4, 32], dt.float32, tag=f"rec1h{lb}")
                nc.vector.reciprocal(rec1h[:], den1h[:])
                nc.vector.tensor_mul(
                    attb1[:, sl].rearrange("p (g j) -> p g j", j=16),
                    e1h[:].rearrange("p (g j) -> p g j", j=16),
                    rec1h[:].unsqueeze(2).broadcast_to([4, 32, 16]),
                )
                # L1 stage-1 for this half's 4 x1-tiles; single striped
                # PSUM group within one ps_s1 buffer (one bank)
                y1p = ps_s1.tile([128, 2, 128], dt.float32, tag="ybps", name="y1p")
                for jl in range(4):
                    j = 4 * lb + jl
                    atp1 = ps_at.tile([128, 4], dt.float32r, tag="atp")
                    nc.tensor.transpose(
                        atp1[:],
                        attb1[0:4, 128 * j : 128 * (j + 1)],
                        id4_t[:],
                    )
                    sab1 = sapool.tile([128, 32], dt.float32r, tag="sab1")
                    nc.vector.tensor_mul(
                        sab1[:].rearrange("p (h g) -> p h g", g=8),
                        f32(atp1[:]).unsqueeze(2).broadcast_to([128, 4, 8]),
                        mm_t[:].rearrange("p (h g) -> p h g", g=8),
                    )
                    for ds in range(2):
                        nc.tensor.matmul(
                            y1p[:, ds, 32 * jl : 32 * jl + 32],
                            x1n_t[:, j, 128 * ds : 128 * (ds + 1)],
                            sab1[:],
                            start=(jl == 0 and ds == 0),
                            stop=(jl == 3 and ds == 1),
                        )
                for ds in range(2):
                    nc.vector.tensor_copy(
                        y1b[ds][:, 128 * lb : 128 * (lb + 1)], y1p[:, ds, :]
                    )
            # 4-stage software pipeline: emission order staggers B(c), C(c-1),
            # D(c-2), E(c-3) so each in-order engine queue holds independent
            # next-chunk work ahead of the dependent tail of older chunks.
            state = {}
            for c in range(NCH + 3):
                if c < NCH:
                    state[c] = emit_B(c)
                if rep == 0 and c == 0:
                    nc.sync.dma_start(
                        out=w0_t[:],
                        in_=w0_d.rearrange("h (fs p) d -> p h fs d", p=128),
                    )
                if rep == 0 and c == 1:
                    nc.sync.dma_start(
                        out=w1_t[:],
                        in_=w1_d.rearrange("h (fs p) d -> p h fs d", p=128),
                    )
                if 0 <= c - 1 < NCH:
                    emit_Cg(c - 1, state[c - 1])
                    emit_Cx(c - 1, state[c - 1])
                if 0 <= c - 2 < NCH:
                    emit_D(c - 2, state[c - 2])
                if 0 <= c - 3 < NCH:
                    emit_E(c - 3, state.pop(c - 3))
                if c == NCH + 1:
                    # light iteration: run layer-1 half 0 (needs E(1) only)
                    emit_L1half(0)
            emit_L1half(1)

            # ---- stage-2 layer 1 + final ----
            out_sb = mpool.tile([64, 256], dt.float32, tag="out_sb", name="out_sb")
            for d2s in range(2):
                ghs1 = []
                for h in range(H):
                    o21 = ps_s2.tile([128, 64], dt.float32, tag="o2", name="o21")
                    for ds in range(2):
                        nc.tensor.matmul(
                            o21[:],
                            w1_t[:, h, ds, 128 * d2s : 128 * (d2s + 1)],
                            y1b[ds][:].rearrange("p (j hh g) -> p j hh g", hh=4, g=8)[
                                :, :, h, :
                            ],
                            start=(ds == 0),
                            stop=(ds == 1),
                        )
                    gh = ghpool.tile([128, 64], dt.float32, tag="gh1", name="gh1")
                    nc.scalar.activation(gh[:], o21[:], AF.Gelu)
                    ghs1.append(gh)
                ad1 = adpool.tile([128, 64], dt.float32, tag="ad1", name="ad1")
                nc.vector.tensor_add(ad1[:], ghs1[0][:], ghs1[1][:])
                ad2 = adpool.tile([128, 64], dt.float32, tag="ad1", name="ad2")
                nc.vector.tensor_add(ad2[:], ghs1[2][:], ghs1[3][:])
                u = mpool.tile([128, 64], dt.float32, tag=f"u{d2s}", name="u")
                nc.vector.tensor_add(u[:], ad1[:], ad2[:])
                uT = mpool.tile([128, 64], dt.float32r, tag=f"uT{d2s}", name="uT")
                nc.vector.tensor_scalar_mul(uT[:], u[:], 0.25)
                otp = ps_at.tile([64, 128], dt.float32r, tag="atp", name="otp")
                nc.tensor.transpose(otp[:], uT[:], id128_t[:])
                nc.vector.tensor_copy(out_sb[:, 128 * d2s : 128 * (d2s + 1)], f32(otp[:]))
                nc.sync.dma_start(
                    out=out_d[:, 128 * d2s : 128 * (d2s + 1)],
                    in_=out_sb[:, 128 * d2s : 128 * (d2s + 1)],
                )

            # ================= LAYER 1 =================
            # epilogue + stage-1 pipelined into emit_E per 512-token half;
            # only stage-2 + final remain here.
    nc.compile()
    return nc


def _prep_weights(W0, A0, W1, A1):
    import ml_dtypes

    def effs(W, A):
        # a_eff[h] = W[h] @ A[h,:256,0]; b_eff[h] = W[h] @ A[h,256:,0]
        a = np.einsum("hfd,hd->hf", W.astype(np.float64), A[:, :256, 0].astype(np.float64))
        b = np.einsum("hfd,hd->hf", W.astype(np.float64), A[:, 256:, 0].astype(np.float64))
        # cols j: 0..3 = a_eff per head, 4..7 = b_eff per head -> [F, 8]
        return np.concatenate([a.T, b.T], axis=1).astype(np.float32)

    ae0 = effs(W0, A0)  # [256, 8]
    ae1 = 0.25 * effs(W1, A1)  # [256, 8]
    aeb = ae0.reshape(2, 128, 8).astype(ml_dtypes.bfloat16)
    ae1r = np.ascontiguousarray(ae1.reshape(2, 128, 8))
    w1s = (0.25 * W1).astype(np.float32)

    t = np.arange(128)
    c = np.arange(32)
    mmask = ((c[None, :] % 8) == (t[:, None] // 16)).astype(np.float32)
    id4 = np.eye(4, dtype=np.float32)
    id128 = np.eye(128, dtype=np.float32)
    return {
        "w0": np.ascontiguousarray(W0.astype(np.float32)),
        "w1": np.ascontiguousarray(w1s),
        "aeb": np.ascontiguousarray(aeb),
        "ae1": ae1r.astype(np.float32),
        "mmask": mmask,
        "id4": id4,
        "id128": id128,
        "id128b": np.eye(128, dtype=ml_dtypes.bfloat16),
    }


def _prep_x(x):
    import ml_dtypes

    return np.ascontiguousarray(np.asarray(x, np.float32).astype(ml_dtypes.bfloat16))


def kernel(x, W0, A0, W1, A1):
    x = np.asarray(x, dtype=np.float32)
    W0 = np.asarray(W0, dtype=np.float32)
    A0 = np.asarray(A0, dtype=np.float32)
    W1 = np.asarray(W1, dtype=np.float32)
    A1 = np.asarray(A1, dtype=np.float32)

    if "nc" not in _CACHE:
        _CACHE["nc"] = build_program()
    nc = _CACHE["nc"]

    wmap = _prep_weights(W0, A0, W1, A1)
    xs = _prep_x(x).reshape(NCORES, T0, F)
    in_maps = [dict(wmap, x=np.ascontiguousarray(xs[i])) for i in range(NCORES)]
    res = run_bass_kernel_spmd(
        nc, in_maps, core_ids=list(range(NCORES)), trace=TRACE
    )
    _CACHE["last_result"] = res
    out = np.concatenate([res.results[i]["out"] for i in range(NCORES)], axis=0)
    return out



# revision 14
# speedup vs baseline: 1.6458x; 1.6458x over previous
"""Trainium2 Bass kernel for a 2-layer GAT-style reduction network.

Reference math (per head h, per group of 16 nodes):
    wx   = x @ W[h]                                  # [*, n, d]
    z    = gelu(wx @ A_top[h] + wx[root] @ A_bot[h]) # root = node 0 of group
    att  = softmax(gelu(z), over n)
    out_h[g] = gelu(sum_n att[n] * wx[n])
    layer out = mean_h out_h
Applied twice: layer0 groups = N1(16) within (b, n2); layer1 groups = N2(16)
within b.

Algebraic restructure:
  - sum_n att[n]*(x[n] @ W) == (sum_n att[n]*x[n]) @ W, so the big matmul
    runs per-group (1024 rows/core) instead of per-node (16384 rows/core).
  - wx @ A_top == x @ (W @ A_top) := x @ a_eff; scores come from thin
    [F, 4] matmuls (a_eff per head), b-scores only at root tokens.
  - head-mean of layer0 absorbed into layer1 weights (0.25 * W1/aeff1);
    final 0.25 applied explicitly.

Distribution: pure data-parallel over 8 NeuronCores (batch 512 -> 64/core).
Weights replicated, bf16. No collectives. x cast to bf16 on the host.

Key structural points vs a naive pipeline (all chosen from measurement:
per-execution overhead is ~212us + ~24us/MB of input bytes, so inputs are
kept minimal; SWDGE SBUF<->SBUF packing DMAs cost ~2.6us each and are
eliminated entirely):
  - a/b attention scores accumulate DIRECTLY in PSUM across all 8 blocks of
    a half-layer via zero-padded stationary weights (aebA/aebB variants with
    the 4 head-columns placed at 4*(b%8)); no score-packing DMAs at all.
    Row layout of the [64, 512] score tile: 32*hf + 4*(b%8) + h, columns =
    tokens within the hf half-block; b-scores ([64, 32]) hold root tokens
    only (one matmul per (block, fs) with a stride-16 moving AP).
  - softmax epilogue runs batched per 8-block half ([64, 512] tiles, full
    ACT lanes), gelu/exp table switches batched (~6 per rep).
  - att transpose for stage-1: ONE PE transpose per 128-token window of the
    [64, 512] att tile ([64, 128] -> [128, 64]); a single DVE mul per block
    builds the masked S_att [128, 8, 32] from the transposed columns.
  - x transposed on-chip per block (PE transpose, 8 per fs into one PSUM
    bank, one [128, 1024] bf16 drain per fs split across DVE/ACT).
  - stage-2 weights + ybuf in bf16 (FWL weight loads, halved drain cost).
  - layer-1 scores use the same zero-padded-accumulation trick per 256-token
    quarter ([16, 256] score tile, rows 4*qt + h).

Avoid: SBUF->SBUF transpose=True DMAs (~300us each on HW) and
partition-crossing multi-dim rearranges in DMA APs (silently scramble data).
"""

import sys

sys.path.insert(0, "/opt/trn_rl_repo")

import numpy as np
from contextlib import ExitStack

import concourse.bass as bass
import concourse.tile as tile
from concourse import bacc, mybir
from concourse.bass_utils import run_bass_kernel_spmd

dt = mybir.dt
AF = mybir.ActivationFunctionType

NCORES = 8
B, N2, N1, F, D, H = 512, 16, 16, 256, 256, 4
BS = B // NCORES  # 64 samples per core
T0 = BS * N2 * N1  # 16384 tokens, layer 0
NB0 = T0 // 1024  # 16 blocks of 1024 tokens
T1 = BS * N2  # 1024 tokens, layer 1

import os as _os

REPS = int(_os.environ.get("KREPS", "1"))
TRACE = False
_CACHE = {}


def f32(ap):
    return ap.bitcast(dt.float32)


def build_program():
    nc = bacc.Bacc("TRN2", target_bir_lowering=False, debug=False)

    x_d = nc.dram_tensor("x", [T0, F], dt.bfloat16, kind="ExternalInput").ap()
    w0_d = nc.dram_tensor("w0", [H, F, D], dt.bfloat16, kind="ExternalInput").ap()
    w1_d = nc.dram_tensor("w1", [H, D, D], dt.bfloat16, kind="ExternalInput").ap()
    aebA_d = nc.dram_tensor("aebA", [8, 2, 128, 32], dt.bfloat16, kind="ExternalInput").ap()
    aebB_d = nc.dram_tensor("aebB", [8, 2, 128, 32], dt.bfloat16, kind="ExternalInput").ap()
    ae1A_d = nc.dram_tensor("ae1A", [4, 2, 128, 16], dt.bfloat16, kind="ExternalInput").ap()
    ae1B_d = nc.dram_tensor("ae1B", [4, 2, 128, 16], dt.bfloat16, kind="ExternalInput").ap()
    mm_d = nc.dram_tensor("mmask", [128, 32], dt.float32, kind="ExternalInput").ap()
    id128_d = nc.dram_tensor("id128", [128, 128], dt.float32r, kind="ExternalInput").ap()
    id128b_d = nc.dram_tensor("id128b", [128, 128], dt.bfloat16, kind="ExternalInput").ap()
    out_d = nc.dram_tensor("out", [BS, D], dt.float32, kind="ExternalOutput").ap()

    with tile.TileContext(nc) as tc, ExitStack() as ctx:
        cpool = ctx.enter_context(tc.tile_pool(name="consts", bufs=1))
        xbpool = ctx.enter_context(tc.tile_pool(name="xb", bufs=8))
        xtpool = ctx.enter_context(tc.tile_pool(name="xt", bufs=9))
        attpool = ctx.enter_context(tc.tile_pool(name="att", bufs=2))
        epool = ctx.enter_context(tc.tile_pool(name="eps", bufs=2))
        sapool = ctx.enter_context(tc.tile_pool(name="sab", bufs=3))
        ybpool = ctx.enter_context(tc.tile_pool(name="ybuf", bufs=2))
        ghpool = ctx.enter_context(tc.tile_pool(name="gh", bufs=4))
        adpool = ctx.enter_context(tc.tile_pool(name="ad", bufs=2))
        x1pool = ctx.enter_context(tc.tile_pool(name="x1", bufs=1))
        mpool = ctx.enter_context(tc.tile_pool(name="misc", bufs=1))

        # PSUM: 4 pools x 2 slots x 1 bank = 8 banks exactly.
        #   ps_big: x-transpose staging banks (B) -> stage-2 o2 (E) -> L1 o21/otp
        #   ps_z:   zaps_q [64, 512] (B/C) -> L1 z1a
        #   ps_r:   zbr_q [64, 32] (B/C) -> attT atp_q (D) -> L1 zbr1/atp1
        #   ps_s1:  stage-1 ybps -> L1 y1p
        ps_big = ctx.enter_context(tc.tile_pool(name="ps_big", bufs=2, space="PSUM"))
        ps_z = ctx.enter_context(tc.tile_pool(name="ps_z", bufs=2, space="PSUM"))
        ps_r = ctx.enter_context(tc.tile_pool(name="ps_r", bufs=2, space="PSUM"))
        ps_s1 = ctx.enter_context(tc.tile_pool(name="ps_s1", bufs=2, space="PSUM"))

        # ---- constants (scalar queue; x loads own the sync queue) ----
        w0_t = cpool.tile([128, H, 2, D], dt.bfloat16, tag="w0")
        w1_t = cpool.tile([128, H, 2, D], dt.bfloat16, tag="w1")
        aebA_t = cpool.tile([128, 8, 2, 32], dt.bfloat16, tag="aebA")
        nc.scalar.dma_start(out=aebA_t[:], in_=aebA_d.rearrange("b s p j -> p b s j"))
        aebB_t = cpool.tile([128, 8, 2, 32], dt.bfloat16, tag="aebB")
        nc.scalar.dma_start(out=aebB_t[:], in_=aebB_d.rearrange("b s p j -> p b s j"))
        ae1A_t = cpool.tile([128, 4, 2, 16], dt.bfloat16, tag="ae1A")
        nc.scalar.dma_start(out=ae1A_t[:], in_=ae1A_d.rearrange("q s p j -> p q s j"))
        ae1B_t = cpool.tile([128, 4, 2, 16], dt.bfloat16, tag="ae1B")
        nc.scalar.dma_start(out=ae1B_t[:], in_=ae1B_d.rearrange("q s p j -> p q s j"))
        mm_t = cpool.tile([128, 32], dt.float32, tag="mm")
        nc.scalar.dma_start(out=mm_t[:], in_=mm_d)
        id128_t = cpool.tile([128, 128], dt.float32r, tag="id128")
        nc.scalar.dma_start(out=id128_t[:], in_=id128_d)
        id128b_t = cpool.tile([128, 128], dt.bfloat16, tag="id128b")
        nc.scalar.dma_start(out=id128b_t[:], in_=id128b_d)

        for rep in range(REPS):
            # ============ LAYER 0 ============
            x1T = x1pool.tile([128, 2, 1024], dt.bfloat16, tag="x1T", name="x1T")
            x1n = x1pool.tile([128, 8, 256], dt.bfloat16, tag="x1n", name="x1n")

            xbs = {}  # pair index -> [128, 16, 256] bf16 (2 blocks)
            xtbs = {}  # block -> [128, 2, 1024] bf16 transposed x
            zaps = {}  # q -> [64, 512] psum, rows 32*hf + 4*b8 + h
            zbrp = {}  # q -> [64, 32] psum, root b-scores per group
            att_sb = {}  # q -> [64, 512] f32r att weights
            atp = {}  # q -> [128, 4, 64] f32r transposed att (psum)
            ndr = [0]

            def emit_xload(pp):
                xb = xbpool.tile([128, 16, 256], dt.bfloat16, tag="xb", name="xb")
                nc.sync.dma_start(
                    out=xb[:],
                    in_=x_d[2048 * pp : 2048 * (pp + 1), :].rearrange(
                        "(n p) f -> p n f", p=128
                    ),
                )
                xbs[pp] = xb

            def emit_Bt(b):
                # PE-transpose block b: 8 [128,128] transposes per fs into one
                # PSUM bank, one [128, 1024] bf16 drain per fs (DVE/ACT split)
                xb = xbs[b // 2]
                ko = 8 * (b % 2)
                xtb = xtpool.tile([128, 2, 1024], dt.bfloat16, tag="xt", name="xtb")
                for fs in range(2):
                    tp = ps_big.tile([128, 1024], dt.bfloat16, tag="big", name="tp")
                    for k in range(8):
                        nc.tensor.matmul(
                            tp[:, 128 * k : 128 * (k + 1)],
                            xb[:, ko + k, 128 * fs : 128 * (fs + 1)],
                            id128b_t[:],
                            is_transpose=True,
                        )
                    if ndr[0] % 8 < 5:
                        nc.vector.tensor_copy(xtb[:, fs, :], tp[:])
                    else:
                        nc.scalar.copy(xtb[:, fs, :], tp[:])
                    ndr[0] += 1
                xtbs[b] = xtb

            def emit_scores(q, hf):
                # one strictly-sequential accumulation group per (bank, hf):
                # all 8 blocks x 2 fs accumulate via zero-padded stationaries
                if hf == 0:
                    zaps[q] = ps_z.tile([64, 512], dt.float32, tag="zq", name="zaps")
                    zbrp[q] = ps_r.tile([64, 32], dt.float32, tag="rq", name="zbrp")
                for b8 in range(8):
                    xtb = xtbs[8 * q + b8]
                    for fs in range(2):
                        first = b8 == 0 and fs == 0
                        last = b8 == 7 and fs == 1
                        # skip_group_check: the sim's zero-region tracker
                        # mis-addresses partition-offset outputs; the two hf
                        # groups per bank are strictly sequential by design.
                        nc.tensor.matmul(
                            zaps[q][32 * hf : 32 * (hf + 1), :],
                            aebA_t[:, b8, fs, :],
                            xtb[:, fs, 512 * hf : 512 * (hf + 1)],
                            start=first,
                            stop=last,
                            skip_group_check=True,
                        )
                        nc.tensor.matmul(
                            zbrp[q][32 * hf : 32 * (hf + 1), :],
                            aebB_t[:, b8, fs, :],
                            xtb[:, fs, 512 * hf : 512 * (hf + 1)].rearrange(
                                "p (g j) -> p g j", j=16
                            )[:, :, 0:1],
                            start=first,
                            stop=last,
                            skip_group_check=True,
                        )
                if hf == 1:
                    for b8 in range(8):
                        xtbs.pop(8 * q + b8)

            def emit_C(q):
                # softmax epilogue on [64, 512]: rows (hf, b8, h), cols tokens
                zbr_sb = mpool.tile([64, 32], dt.float32, tag=f"zbr{q}", name="zbr_sb")
                nc.scalar.copy(zbr_sb[:], zbrp[q][:])
                zs = epool.tile([64, 512], dt.float32, tag="epsA", name="zs")
                nc.vector.tensor_add(
                    zs[:].rearrange("p (g j) -> p g j", j=16),
                    zaps[q][:].rearrange("p (g j) -> p g j", j=16),
                    zbr_sb[:].unsqueeze(2).broadcast_to([64, 32, 16]),
                )
                g1 = epool.tile([64, 512], dt.float32, tag="epsB", name="g1")
                nc.scalar.activation(g1[:], zs[:], AF.Gelu)
                sv = epool.tile([64, 512], dt.float32, tag="epsA", name="sv")
                nc.scalar.activation(sv[:], g1[:], AF.Gelu)
                e = epool.tile([64, 512], dt.float32, tag="epsB", name="e")
                nc.scalar.activation(e[:], sv[:], AF.Exp)
                den = mpool.tile([64, 32], dt.float32, tag=f"den{q}", name="den")
                nc.vector.reduce_sum(
                    den[:].unsqueeze(2),
                    e[:].rearrange("p (g j) -> p g j", j=16),
                    axis=mybir.AxisListType.X,
                )
                rec = mpool.tile([64, 32], dt.float32, tag=f"rec{q}", name="rec")
                nc.vector.reciprocal(rec[:], den[:])
                att = attpool.tile([64, 512], dt.float32r, tag="att", name="att")
                nc.vector.tensor_mul(
                    att[:].rearrange("p (g j) -> p g j", j=16),
                    e[:].rearrange("p (g j) -> p g j", j=16),
                    rec[:].unsqueeze(2).broadcast_to([64, 32, 16]),
                )
                att_sb[q] = att

            def emit_attT(q):
                # one transpose per 128-token window: [64, 128] -> [128, 64]
                a = ps_r.tile([128, 4, 64], dt.float32r, tag="rq", name="atp")
                for w in range(4):
                    nc.tensor.transpose(
                        a[:, w, :],
                        att_sb[q][:, 128 * w : 128 * (w + 1)],
                        id128_t[0:64, 0:64],
                    )
                atp[q] = a

            ybps = [None, None]

            def emit_D(c, ybuf):
                # stage-1 for blocks 4c..4c+4: S_att build + x-stationary matmul
                q = c // 2
                for bl in range(4):
                    b = 4 * c + bl
                    b8 = b % 8
                    sab = sapool.tile([128, 8, 32], dt.bfloat16, tag="sab", name="sab")
                    # sab[p, (hf,w), (h,g)] = atp[q][p, w, 32*hf+4*b8+h] * mask[p, g]
                    for hf in range(2):
                        nc.vector.tensor_mul(
                            sab[:, 4 * hf : 4 * (hf + 1), :].rearrange(
                                "p w (hh g) -> p w hh g", hh=4
                            ),
                            f32(atp[q][:])[:, :, 32 * hf + 4 * b8 : 32 * hf + 4 * b8 + 4]
                            .unsqueeze(3)
                            .broadcast_to([128, 4, 4, 8]),
                            mm_t[:]
                            .rearrange("p (hh g) -> p hh g", g=8)
                            .unsqueeze(1)
                            .broadcast_to([128, 4, 4, 8]),
                        )
                    xb = xbs[b // 2]
                    ko = 8 * (b % 2)
                    for k in range(8):
                        K = b * 8 + k
                        kq = K % 16
                        if kq == 0:
                            ybps[0] = ps_s1.tile([128, 512], dt.float32, tag="s1", name="yb0")
                            ybps[1] = ps_s1.tile([128, 512], dt.float32, tag="s1", name="yb1")
                        for fs in range(2):
                            nc.tensor.matmul(
                                ybps[fs][:, 32 * kq : 32 * kq + 32],
                                xb[:, ko + k, 128 * fs : 128 * (fs + 1)],
                                sab[:, k, :],
                                start=(kq == 0),
                                stop=(kq == 15),
                            )
                        if kq == 15:
                            qq = (K % 32) // 16
                            nc.scalar.copy(
                                ybuf[:, 0, 512 * qq : 512 * (qq + 1)], ybps[0][:]
                            )
                            nc.vector.tensor_copy(
                                ybuf[:, 1, 512 * qq : 512 * (qq + 1)], ybps[1][:]
                            )

            def emit_E(c, ybuf):
                # stage-2 + gelu head-sum -> x1T cols 256c:256(c+1)
                ghs = []
                for h in range(H):
                    o2 = ps_big.tile([128, 512], dt.float32, tag="big", name="o2")
                    for ds in range(2):
                        for fs in range(2):
                            nc.tensor.matmul(
                                o2[:, 256 * ds : 256 * (ds + 1)],
                                w0_t[:, h, fs, 128 * ds : 128 * (ds + 1)],
                                ybuf[:, fs, :].rearrange(
                                    "p (K hh g) -> p K hh g", hh=4, g=8
                                )[:, :, h, :],
                                start=(ds == 0 and fs == 0),
                                stop=(ds == 1 and fs == 1),
                            )
                    gh = ghpool.tile([128, 512], dt.bfloat16, tag="gh", name="gh")
                    nc.scalar.activation(gh[:], o2[:], AF.Gelu)
                    ghs.append(gh)
                ad1 = adpool.tile([128, 512], dt.bfloat16, tag="ad", name="ad1")
                nc.vector.tensor_add(ad1[:], ghs[0][:], ghs[1][:])
                ad2 = adpool.tile([128, 512], dt.bfloat16, tag="ad", name="ad2")
                nc.vector.tensor_add(ad2[:], ghs[2][:], ghs[3][:])
                nc.vector.tensor_add(
                    x1T[:, :, 256 * c : 256 * (c + 1)],
                    ad1[:].rearrange("p (ds i) -> p ds i", ds=2),
                    ad2[:].rearrange("p (ds i) -> p ds i", ds=2),
                )

            def emit_x1n():
                # layer-1 prep: transpose x1T columns to token-major x1n
                for j in range(8):
                    for ds in range(2):
                        trp = ps_r.tile([128, 128], dt.bfloat16, tag="rq", name="trp")
                        nc.tensor.transpose(
                            trp[:],
                            x1T[:, ds, 128 * j : 128 * (j + 1)],
                            id128b_t[:],
                        )
                        nc.vector.tensor_copy(
                            x1n[:, j, 128 * ds : 128 * (ds + 1)], trp[:]
                        )

            # ---- emission schedule ----
            for pp in range(4):
                emit_xload(pp)
            for b in range(8):
                emit_Bt(b)
                if b == 1:
                    for pp in range(4, 8):
                        emit_xload(pp)
                if rep == 0 and b == 2:
                    nc.scalar.dma_start(
                        out=w0_t[:], in_=w0_d.rearrange("h (fs p) d -> p h fs d", p=128)
                    )
                if rep == 0 and b == 4:
                    nc.scalar.dma_start(
                        out=w1_t[:], in_=w1_d.rearrange("h (fs p) d -> p h fs d", p=128)
                    )
            emit_scores(0, 0)
            emit_scores(0, 1)
            emit_C(0)
            for b in range(8, 16):
                emit_Bt(b)
            emit_scores(1, 0)
            emit_scores(1, 1)
            emit_attT(0)
            ybufs = {}
            ybufs[0] = ybpool.tile([128, 2, 1024], dt.bfloat16, tag="ybuf", name="ybuf")
            emit_D(0, ybufs[0])
            ybufs[1] = ybpool.tile([128, 2, 1024], dt.bfloat16, tag="ybuf", name="ybuf")
            emit_D(1, ybufs[1])
            emit_C(1)
            emit_attT(1)
            emit_E(0, ybufs[0])
            ybufs[2] = ybpool.tile([128, 2, 1024], dt.bfloat16, tag="ybuf", name="ybuf")
            emit_D(2, ybufs[2])
            emit_E(1, ybufs[1])
            ybufs[3] = ybpool.tile([128, 2, 1024], dt.bfloat16, tag="ybuf", name="ybuf")
            emit_D(3, ybufs[3])
            emit_E(2, ybufs[2])
            emit_E(3, ybufs[3])
            emit_x1n()

            # ============ LAYER 1 ============
            # scores per 256-token quarter, rows 4*qt + h, zero-padded accum
            z1a = ps_z.tile([16, 256], dt.float32, tag="zq", name="z1a")
            zbr1 = ps_r.tile([16, 16], dt.float32, tag="rq", name="zbr1")
            for qt in range(4):
                for ds in range(2):
                    nc.tensor.matmul(
                        z1a[:],
                        ae1A_t[:, qt, ds, :],
                        x1T[:, ds, 256 * qt : 256 * (qt + 1)],
                        start=(qt == 0 and ds == 0),
                        stop=(qt == 3 and ds == 1),
                    )
                    nc.tensor.matmul(
                        zbr1[:],
                        ae1B_t[:, qt, ds, :],
                        x1T[:, ds, 256 * qt : 256 * (qt + 1)].rearrange(
                            "p (g j) -> p g j", j=16
                        )[:, :, 0:1],
                        start=(qt == 0 and ds == 0),
                        stop=(qt == 3 and ds == 1),
                    )
            zbr1_sb = mpool.tile([16, 16], dt.float32, tag="zbr1s", name="zbr1_sb")
            nc.scalar.copy(zbr1_sb[:], zbr1[:])
            zs1 = epool.tile([16, 256], dt.float32, tag="epsA", name="zs1")
            nc.vector.tensor_add(
                zs1[:].rearrange("p (g j) -> p g j", j=16),
                z1a[:].rearrange("p (g j) -> p g j", j=16),
                zbr1_sb[:].unsqueeze(2).broadcast_to([16, 16, 16]),
            )
            g11 = epool.tile([16, 256], dt.float32, tag="epsB", name="g11")
            nc.scalar.activation(g11[:], zs1[:], AF.Gelu)
            s1t = epool.tile([16, 256], dt.float32, tag="epsA", name="s1t")
            nc.scalar.activation(s1t[:], g11[:], AF.Gelu)
            e1 = epool.tile([16, 256], dt.float32, tag="epsB", name="e1")
            nc.scalar.activation(e1[:], s1t[:], AF.Exp)
            den1 = mpool.tile([16, 16], dt.float32, tag="den1", name="den1")
            nc.vector.reduce_sum(
                den1[:].unsqueeze(2),
                e1[:].rearrange("p (g j) -> p g j", j=16),
                axis=mybir.AxisListType.X,
            )
            rec1 = mpool.tile([16, 16], dt.float32, tag="rec1", name="rec1")
            nc.vector.reciprocal(rec1[:], den1[:])
            att1 = attpool.tile([16, 256], dt.float32r, tag="att1", bufs=1, name="att1")
            nc.vector.tensor_mul(
                att1[:].rearrange("p (g j) -> p g j", j=16),
                e1[:].rearrange("p (g j) -> p g j", j=16),
                rec1[:].unsqueeze(2).broadcast_to([16, 16, 16]),
            )
            # transpose att1: [16, 128] windows -> [128, 2, 16]
            atp1 = ps_r.tile([128, 2, 16], dt.float32r, tag="rq", name="atp1")
            for w in range(2):
                nc.tensor.transpose(
                    atp1[:, w, :],
                    att1[:, 128 * w : 128 * (w + 1)],
                    id128_t[0:16, 0:16],
                )
            sab1 = []
            for w in range(2):
                sb = sapool.tile([128, 4, 32], dt.bfloat16, tag="sab1", name="sab1")
                nc.vector.tensor_mul(
                    sb[:].rearrange("p qt (hh g) -> p qt hh g", g=8),
                    f32(atp1[:, w, :])
                    .rearrange("p (qt hh) -> p qt hh", hh=4)
                    .unsqueeze(3)
                    .broadcast_to([128, 4, 4, 8]),
                    mm_t[:]
                    .rearrange("p (hh g) -> p hh g", g=8)
                    .unsqueeze(1)
                    .broadcast_to([128, 4, 4, 8]),
                )
                sab1.append(sb)
            # stage-1: single accumulation bank [128, 2, 256]
            y1p = ps_s1.tile([128, 2, 256], dt.float32, tag="s1", name="y1p")
            for k1 in range(8):
                qt, w = k1 // 2, k1 % 2
                for ds in range(2):
                    nc.tensor.matmul(
                        y1p[:, ds, 32 * k1 : 32 * k1 + 32],
                        x1n[:, k1, 128 * ds : 128 * (ds + 1)],
                        sab1[w][:, qt, :],
                        start=(k1 == 0 and ds == 0),
                        stop=(k1 == 7 and ds == 1),
                    )
            y1b = mpool.tile([128, 2, 256], dt.bfloat16, tag="y1b", name="y1b")
            nc.vector.tensor_copy(y1b[:], y1p[:])

            # stage-2 + final output
            out_sb = mpool.tile([64, 256], dt.float32, tag="out_sb", name="out_sb")
            for d2s in range(2):
                ghs1 = []
                for hp in range(2):
                    o21 = ps_big.tile([128, 2, 64], dt.float32, tag="big", name="o21")
                    for hh in range(2):
                        h = 2 * hp + hh
                        for ds in range(2):
                            nc.tensor.matmul(
                                o21[:, hh, :],
                                w1_t[:, h, ds, 128 * d2s : 128 * (d2s + 1)],
                                y1b[:, ds, :].rearrange(
                                    "p (j hh g) -> p j hh g", hh=4, g=8
                                )[:, :, h, :],
                                start=(ds == 0),
                                stop=(ds == 1),
                            )
                    gh = ghpool.tile([128, 2, 64], dt.float32, tag="gh1", name="gh1")
                    nc.scalar.activation(gh[:], o21[:], AF.Gelu)
                    ghs1.append(gh)
                ad1 = adpool.tile([128, 64], dt.float32, tag="ad1", name="ad1")
                nc.vector.tensor_add(ad1[:], ghs1[0][:, 0, :], ghs1[0][:, 1, :])
                ad2 = adpool.tile([128, 64], dt.float32, tag="ad1", name="ad2")
                nc.vector.tensor_add(ad2[:], ghs1[1][:, 0, :], ghs1[1][:, 1, :])
                u = mpool.tile([128, 64], dt.float32, tag=f"u{d2s}", name="u")
                nc.vector.tensor_add(u[:], ad1[:], ad2[:])
                uT = mpool.tile([128, 64], dt.float32r, tag=f"uT{d2s}", name="uT")
                nc.vector.tensor_scalar_mul(uT[:], u[:], 0.25)
                otp = ps_big.tile([64, 128], dt.float32r, tag="big", name="otp")
                nc.tensor.transpose(otp[:], uT[:], id128_t[:])
                nc.vector.tensor_copy(out_sb[:, 128 * d2s : 128 * (d2s + 1)], f32(otp[:]))
                nc.scalar.dma_start(
                    out=out_d[:, 128 * d2s : 128 * (d2s + 1)],
                    in_=out_sb[:, 128 * d2s : 128 * (d2s + 1)],
                )
    nc.compile()
    return nc


def _prep_weights(W0, A0, W1, A1):
    import ml_dtypes

    def effs(W, A):
        # a_eff[h] = W[h] @ A[h,:256,0]; b_eff[h] = W[h] @ A[h,256:,0]  -> [F, H]
        a = np.einsum("hfd,hd->hf", W.astype(np.float64), A[:, :256, 0].astype(np.float64))
        b = np.einsum("hfd,hd->hf", W.astype(np.float64), A[:, 256:, 0].astype(np.float64))
        return a.T.astype(np.float32), b.T.astype(np.float32)

    a0, b0 = effs(W0, A0)  # [256, 4] each
    a1, b1 = effs(W1, A1)
    a1, b1 = 0.25 * a1, 0.25 * b1

    def padvar(eff, nv, dtype):
        # eff [256, 4] -> [nv, 2, 128, 4*nv...32/16] with cols 4v..4v+4 = eff
        w = 4 * nv
        out = np.zeros((nv, 2, 128, w), np.float32)
        for v in range(nv):
            out[v, 0, :, 4 * v : 4 * v + 4] = eff[:128]
            out[v, 1, :, 4 * v : 4 * v + 4] = eff[128:]
        return np.ascontiguousarray(out.astype(dtype))

    import ml_dtypes as md

    t = np.arange(128)
    c = np.arange(32)
    mmask = ((c[None, :] % 8) == (t[:, None] // 16)).astype(np.float32)
    return {
        "w0": np.ascontiguousarray(W0.astype(md.bfloat16)),
        "w1": np.ascontiguousarray((0.25 * W1).astype(md.bfloat16)),
        "aebA": padvar(a0, 8, md.bfloat16),
        "aebB": padvar(b0, 8, md.bfloat16),
        "ae1A": padvar(a1, 4, md.bfloat16),
        "ae1B": padvar(b1, 4, md.bfloat16),
        "mmask": mmask,
        "id128": np.eye(128, dtype=np.float32),
        "id128b": np.eye(128, dtype=md.bfloat16),
    }


def _prep_x(x):
    import ml_dtypes

    return np.ascontiguousarray(np.asarray(x, np.float32).astype(ml_dtypes.bfloat16))


def kernel(x, W0, A0, W1, A1):
    x = np.asarray(x, dtype=np.float32)
    W0 = np.asarray(W0, dtype=np.float32)
    A0 = np.asarray(A0, dtype=np.float32)
    W1 = np.asarray(W1, dtype=np.float32)
    A1 = np.asarray(A1, dtype=np.float32)

    if "nc" not in _CACHE:
        _CACHE["nc"] = build_program()
    nc = _CACHE["nc"]

    wmap = _prep_weights(W0, A0, W1, A1)
    xs = _prep_x(x).reshape(NCORES, T0, F)
    in_maps = [dict(wmap, x=np.ascontiguousarray(xs[i])) for i in range(NCORES)]
    res = run_bass_kernel_spmd(
        nc, in_maps, core_ids=list(range(NCORES)), trace=TRACE
    )
    _CACHE["last_result"] = res
    out = np.concatenate([res.results[i]["out"] for i in range(NCORES)], axis=0)
    return out
